# revision 27
# baseline (speedup 1.0000x reference)
"""AnchorTransformer kernel for 8 TRN2 NeuronCores.

Data-parallel over the flattened pixel dim N = B*H*W = 32768 -> 4096/core
(the sharding hint: shard features/instances, replicate anchors + K/V
tables + weights).

Math (per pixel n with instance index i = max(lab-1, 0)):
    q = f W_q^T + b_q
    S[n, j] = scale * q . K_all[j]  over all J=512 anchor rows (64 inst x 8)
    masked softmax over the 8 columns of instance i, attn @ V rows,
    out-proj, background zeroing, residual.

Replicated tables (host-folded, per the sharding hint):
    KW   = scale * (A W_k^T + b_k) W_q          (J, C)  score weights
    sb_j = scale * (A W_k^T + b_k)_j . b_q      (J,)    score bias (exp bias)
    V2   = (A W_v^T + b_v) W_o^T + 1 (x) b_o    (J, C)  out-proj folded into V
           (valid because attention weights sum to 1), plus a ones column
           so the attention matmul also emits the softmax denominator.
    mask = +30 on the 8 selected columns via one-hot(inst) matmul; softmax
           shift-invariance makes this equivalent to -inf masking, with
           e^-30 leakage ~ 1e-13.

Device per 512-pixel block: 12 score matmuls (bf16, f32 PSUM), fused
exp-with-bias on ScalarE, 16 attention matmuls, then one DVE op per
128-pixel sub-tile for normalize+gate+residual.
"""

import numpy as np
import ml_dtypes
import concourse.bass as bass
import concourse.tile as tile
from concourse import bacc, mybir
from concourse.bass_utils import run_bass_kernel_spmd
NCORES = 8
N_FULL = 32768
NP = N_FULL // NCORES  # 4096 pixels per core
C = 256
M = 64
L = 8
J = M * L  # 512
TP = 512   # pixels per block
NMT = NP // TP  # 8
F32 = mybir.dt.float32
BF16 = mybir.dt.bfloat16
SCALE = 1.0 / 16.0
BIG = 30.0

AF = mybir.ActivationFunctionType
OP = mybir.AluOpType


def build_nc():
    from contextlib import ExitStack

    nc = bacc.Bacc()
    fT = nc.declare_dram_parameter("fT", [C, NP], BF16, isOutput=False)
    fpm = nc.declare_dram_parameter("fpm", [NMT, 128, 4 * C], F32, isOutput=False)
    ET = nc.declare_dram_parameter("ET", [128, NP], BF16, isOutput=False)
    gate = nc.declare_dram_parameter("gate", [NMT, 128, 4], F32, isOutput=False)
    KWT = nc.declare_dram_parameter("KWT", [C, J], BF16, isOutput=False)
    sbj = nc.declare_dram_parameter("sbj", [128, 4], F32, isOutput=False)
    V2 = nc.declare_dram_parameter("V2", [J, C + 1], BF16, isOutput=False)
    R30 = nc.declare_dram_parameter("R30", [128, J], BF16, isOutput=False)
    out = nc.declare_dram_parameter("out", [NMT, 128, 4 * C], F32, isOutput=True)

    with tile.TileContext(nc) as tc, ExitStack() as es:
        cp = es.enter_context(tc.tile_pool(name="const", bufs=1))
        io = es.enter_context(tc.tile_pool(name="io", bufs=4))
        sps = es.enter_context(tc.tile_pool(name="sps", space="PSUM", bufs=6))
        ops = es.enter_context(tc.tile_pool(name="ops", space="PSUM", bufs=2))

        # KWT first: it is the warmup weight and the first real dependency
        KWT_sb = []
        for et in range(2):
            t = cp.tile([128, J], BF16, tag=f"kwt{et}")
            (nc.sync if et == 0 else nc.scalar).dma_start(
                t[:], KWT[et * 128:(et + 1) * 128, :])
            KWT_sb.append(t)

        # PE warmup burst: runs while block-0 inputs are still in flight so
        # HAM reaches K=8/8 before the real matmuls; sink DMA defeats DCE.
        wps = sps.tile([128, J], F32, tag="s", bufs=6)
        for w in range(9):
            nc.tensor.matmul(wps[:], KWT_sb[0][:, 0:128], KWT_sb[0][:],
                             start=True, stop=True, skip_group_check=True)
        wsink = io.tile([128, 1], F32, tag="wsink")
        nc.vector.tensor_copy(wsink[:], wps[:, 0:1])
        warm_dram = nc.dram_tensor("warm_sink", [128, 1], F32)
        nc.sync.dma_start(warm_dram[:, :], wsink[:])

        # remaining replicated tables: tiny, needed by exp/mask/O of block 0
        R30_sb = cp.tile([128, J], BF16, tag="r30")
        nc.sync.dma_start(R30_sb[:], R30[:, :])
        sbj_sb = cp.tile([128, 4], F32, tag="sbj")
        nc.scalar.dma_start(sbj_sb[:], sbj[:, :])
        V2_sb = []
        for jt in range(4):
            t = cp.tile([128, C + 1], BF16, tag=f"v2_{jt}")
            (nc.sync if jt % 2 == 0 else nc.scalar).dma_start(
                t[:], V2[jt * 128:(jt + 1) * 128, :])
            V2_sb.append(t)

        def load_inputs(mt):
            fT_t = []
            for et in range(2):
                t = io.tile([128, TP], BF16, tag=f"ft{et}", bufs=4)
                (nc.sync if et == 0 else nc.scalar).dma_start(
                    t[:], fT[et * 128:(et + 1) * 128, mt * TP:(mt + 1) * TP])
                fT_t.append(t)
            ET_t = io.tile([128, TP], BF16, tag="et", bufs=4)
            nc.scalar.dma_start(ET_t[:], ET[:, mt * TP:(mt + 1) * TP])
            gate_t = io.tile([128, 4], F32, tag="gate", bufs=4)
            nc.scalar.dma_start(gate_t[:], gate[mt, :, :])
            fpm_t = io.tile([128, 4 * C], F32, tag="fpm", bufs=4)
            nc.sync.dma_start(fpm_t[:], fpm[mt, :, :])
            return fT_t, ET_t, gate_t, fpm_t

        pending = [load_inputs(0), load_inputs(1)]

        for mt in range(NMT):
            fT_t, ET_t, gate_t, fpm_t = pending[0]
            pending.pop(0)
            if mt + 2 < NMT:
                pending.append(load_inputs(mt + 2))

            sp_l, P_t = [], []
            for jt in range(4):
                sp = sps.tile([128, TP], F32, tag="s", bufs=6)
                nc.tensor.matmul(
                    sp[:], KWT_sb[0][:, jt * 128:(jt + 1) * 128],
                    fT_t[0][:], start=True, stop=False)
                nc.tensor.matmul(
                    sp[:], KWT_sb[1][:, jt * 128:(jt + 1) * 128],
                    fT_t[1][:], start=False, stop=False)
                sp_l.append(sp)
            # block-diagonal mask: 4 K=32 matmuls packed into distinct PE
            # row-groups -- they run concurrently (~one matmul time total)
            for jt in range(4):
                nc.tensor.matmul(
                    sp_l[jt][:],
                    R30_sb[32 * jt:32 * jt + 32, jt * 128:(jt + 1) * 128],
                    ET_t[32 * jt:32 * jt + 32, :],
                    start=False, stop=True, tile_position=(32 * jt, 0))
            for jt in range(4):
                pt = io.tile([128, TP], BF16, tag=f"p{jt}", bufs=3)
                nc.scalar.activation(pt[:], sp_l[jt][:], AF.Exp,
                                     bias=sbj_sb[:, jt:jt + 1])
                P_t.append(pt)

            otb = io.tile([128, 4 * C], F32, tag="otb", bufs=3)
            for st in range(4):
                op = ops.tile([128, C + 1], F32, tag="o", bufs=2)
                for jt in range(4):
                    nc.tensor.matmul(
                        op[:], P_t[jt][:, st * 128:(st + 1) * 128],
                        V2_sb[jt][:], start=(jt == 0), stop=(jt == 3))
                recip = io.tile([128, 1], F32, tag="recip", bufs=4)
                nc.vector.reciprocal(recip[:], op[:, C:C + 1])
                rg = io.tile([128, 1], F32, tag="rg", bufs=4)
                nc.vector.tensor_mul(rg[:], recip[:], gate_t[:, st:st + 1])
                nc.vector.scalar_tensor_tensor(
                    otb[:, st * C:(st + 1) * C], op[:, 0:C], rg[:, 0:1],
                    fpm_t[:, st * C:(st + 1) * C], OP.mult, OP.add)
            nc.sync.dma_start(out[mt, :, :], otb[:])

    nc.compile()
    return nc


_CACHE = {}


def _build():
    if "nc" not in _CACHE:
        _CACHE["nc"] = build_nc()
    return _CACHE["nc"]


def _prep_maps(anchors, features, instances_in_view, in_proj_w, in_proj_b,
               out_w, out_b):
    f32 = np.float32
    bf16 = ml_dtypes.bfloat16
    anchors = np.asarray(anchors, f32)
    features = np.asarray(features, f32)
    iiv = np.asarray(instances_in_view, np.int32)
    in_proj_w = np.asarray(in_proj_w, f32)
    in_proj_b = np.asarray(in_proj_b, f32)
    out_w = np.asarray(out_w, f32)
    out_b = np.asarray(out_b, f32)

    f_flat = features.reshape(N_FULL, C)
    fT_full = np.ascontiguousarray(f_flat.T.astype(bf16))
    lab = iiv.reshape(-1)
    idx = np.maximum(lab - 1, 0)
    # per j-tile jt, one-hot rows for instances [16jt,16jt+16) live at
    # partitions [32jt, 32jt+16) so K=32 mask matmuls are 32-aligned
    ET_full = np.zeros((128, N_FULL), bf16)
    for jt in range(4):
        rows = (idx[None, :] == (16 * jt + np.arange(16, dtype=np.int32))[:, None])
        ET_full[32 * jt:32 * jt + 16, :] = rows.astype(bf16)
    gate_full = (lab > 0).astype(f32)

    # replicated K/V tables (see module docstring)
    A = anchors.reshape(J, C)
    Wq, Wk, Wv = in_proj_w[:C], in_proj_w[C:2 * C], in_proj_w[2 * C:]
    bq, bk, bv = in_proj_b[:C], in_proj_b[C:2 * C], in_proj_b[2 * C:]
    K_all = A @ Wk.T + bk                      # (J, C)
    KW = f32(SCALE) * (K_all @ Wq)             # (J, C)
    sb = f32(SCALE) * (K_all @ bq)             # (J,)
    V2_h = (A @ Wv.T + bv) @ out_w.T + out_b   # (J, C)
    V2aug = np.concatenate([V2_h, np.ones((J, 1), f32)], axis=1).astype(bf16)
    KWT_h = np.ascontiguousarray(KW.T.astype(bf16))      # (C, J)
    sbj_h = np.ascontiguousarray(sb.reshape(4, 128).T)   # (128, 4) j-major
    R30_h = np.zeros((128, J), f32)
    for jt in range(4):
        for r in range(16):
            m = 16 * jt + r
            R30_h[32 * jt + r, m * L:(m + 1) * L] = BIG
    R30_h = R30_h.astype(bf16)

    in_maps = []
    for i in range(NCORES):
        sl = slice(i * NP, (i + 1) * NP)
        in_maps.append({
            "fT": np.ascontiguousarray(fT_full[:, sl]),
            "fpm": np.ascontiguousarray(
                f_flat[sl].reshape(NMT, 4, 128, C).transpose(0, 2, 1, 3)
                .reshape(NMT, 128, 4 * C)),
            "ET": np.ascontiguousarray(ET_full[:, sl]),
            "gate": np.ascontiguousarray(
                gate_full[sl].reshape(NMT, 4, 128).transpose(0, 2, 1)),
            "KWT": KWT_h, "sbj": sbj_h, "V2": V2aug, "R30": R30_h,
        })
    return in_maps, features.shape


def _run(in_maps, **kw):
    nc = _build()
    return run_bass_kernel_spmd(nc, in_maps, core_ids=list(range(NCORES)), **kw)


def kernel(**inputs):
    in_maps, shp = _prep_maps(**inputs)
    res = _run(in_maps)
    outs = [
        np.asarray(r["out"]).reshape(NMT, 128, 4, C).transpose(0, 2, 1, 3)
        .reshape(NP, C)
        for r in res.results
    ]
    return np.concatenate(outs, axis=0).reshape(shp).astype(np.float32)
